# revision 1
# baseline (speedup 1.0000x reference)
"""GCN 2-layer encoder on 8 TRN2 NeuronCores.

Strategy (graph/data parallel, aggregate-first form):
  out = A_hat @ (relu((A_hat @ x) @ W1 + b1) @ W2) + b2
where A_hat = D^-1/2 (A + I) D^-1/2.  Since aggregation is linear it
commutes with the dense layer:  A_hat (x W1) == (A_hat x) W1.

Per core (nodes sharded 8 x 12544 padded rows):
  1. gather x_scaled[src] rows (x pre-scaled by dinv on host) with
     dma_gather (int16 indices, 4 banks of 25088 rows, 4 SWDGE queues),
     one-hot segment-sum matmul per 128-node dst tile (f32r),
     epilogue scales by dinv[dst] -> y = A_hat x
  2. yT via PE transpose; h1T = W1^T yT; relu+bias (ACT);
     h2T = W2^T rT; transpose back -> h2 rows, scaled by dinv[node]
  3. AllGather h2 shards -> full h2 table (51MB DRAM per core)
  4. same gather/segsum in bf16 over h2 -> + b2 -> output shard
Host assembles the 8 shards.
"""
import sys

sys.path.insert(0, "/opt/trn_rl_repo")
import os
import numpy as np
import ml_dtypes

import concourse.bass as bass
import concourse.bacc as bacc
import concourse.mybir as mybir
import concourse.tile as tile
from concourse import bass_utils
from concourse.masks import make_identity

P = 128
NC = 8
N = 100_000
NPAD = 100_352  # 8 * 12544
SHARD = NPAD // NC  # 12544
TD = SHARD // P  # 98 dst tiles per core
NB = 4  # index banks (int16 range)
BANK = NPAD // NB  # 25088
D_IN = 256
H1 = 256
H2 = 128
f32 = mybir.dt.float32
f32r = mybir.dt.float32r
bf16 = mybir.dt.bfloat16
i16 = mybir.dt.int16

LAST_EXEC_NS = None
LAST_RESULT = None
LAST_TB = None


def _pack(edge_src, edge_dst):
    """Sort/pad edges into per-core (group, bank, tile, subtile) slots.

    Group-major layout so one dma_gather covers a whole group's bank runs.
    Returns T_b and per-core dicts of index/selector arrays.
    """
    GRP = 4
    ngrp = (TD + GRP - 1) // GRP
    gsizes = [min(GRP, TD - g * GRP) for g in range(ngrp)]
    cores = []
    maxrun = 0
    percore = []
    for c in range(NC):
        lo, hi = c * SHARD, (c + 1) * SHARD
        sel = (edge_dst >= lo) & (edge_dst < hi)
        s, d = edge_src[sel], edge_dst[sel]
        tile_id = (d - lo) // P
        bank = s // BANK
        key = tile_id * NB + bank
        order = np.argsort(key, kind="stable")
        s, d, key = s[order], d[order], key[order]
        runs = np.bincount(key, minlength=TD * NB)
        maxrun = max(maxrun, int(runs.max()))
        percore.append((s, d, key, runs))
    T_b = (maxrun + P - 1) // P
    T_e = NB * T_b
    # slot base of each (tile, bank) run in group-major order
    grp_of = np.arange(TD) // GRP
    di_of = np.arange(TD) % GRP
    gs_of = np.array([gsizes[g] for g in grp_of])
    grp_base = np.zeros(ngrp, dtype=np.int64)
    for g in range(1, ngrp):
        grp_base[g] = grp_base[g - 1] + NB * gsizes[g - 1] * T_b * P
    nslots = int(grp_base[-1] + NB * gsizes[-1] * T_b * P)
    run_base = np.zeros((TD, NB), dtype=np.int64)
    for d in range(TD):
        g, di, gs = grp_of[d], di_of[d], gs_of[d]
        for b in range(NB):
            run_base[d, b] = grp_base[g] + (b * gs + di) * (T_b * P)
    for c in range(NC):
        s, d, key, runs = percore[c]
        lo = c * SHARD
        gidx = np.zeros(nslots, dtype=np.int16)
        dstl = np.full(nslots, -1.0, dtype=np.float32)
        first = np.zeros(TD * NB, dtype=np.int64)
        first[1:] = np.cumsum(runs)[:-1]
        rank = np.arange(len(key)) - first[key]
        slot = run_base[key // NB, key % NB] + rank
        gidx[slot] = (s % BANK).astype(np.int16)
        dstl[slot] = ((d - lo) % P).astype(np.float32)
        # wrap for dma_gather: per call (grp, bank) = gs*T_b*128 linear idxs
        wrapped_parts = []
        for g in range(ngrp):
            gs = gsizes[g]
            w = gs * T_b * P
            for b in range(NB):
                a = gidx[grp_base[g] + b * w : grp_base[g] + (b + 1) * w]
                wrapped_parts.append(a.reshape(w // 16, 16).T)
        wrapped16 = np.concatenate(wrapped_parts, axis=1)
        wrapped = np.tile(wrapped16, (8, 1))
        # selector cols: [128, TD*T_e], col = d*T_e + b*T_b + j, row q
        dstl_cols = np.zeros((P, TD * T_e), dtype=np.float32)
        for d2 in range(TD):
            g, di, gs = grp_of[d2], di_of[d2], gs_of[d2]
            for b in range(NB):
                blk = dstl[run_base[d2, b] : run_base[d2, b] + T_b * P]
                dstl_cols[:, d2 * T_e + b * T_b : d2 * T_e + (b + 1) * T_b] = (
                    blk.reshape(T_b, P).T
                )
        cores.append(
            {
                "gidx": np.ascontiguousarray(wrapped),
                "dstl_bf": dstl_cols.astype(ml_dtypes.bfloat16),
            }
        )
    return T_b, cores


def _build(T_b):
    T_e = NB * T_b
    nc = bacc.Bacc(
        "TRN2",
        target_bir_lowering=False,
        debug=False,
        num_devices=NC,
        num_swdge_queues=4,
    )
    xs = nc.dram_tensor("xs", [NPAD, D_IN], bf16, kind="ExternalInput").ap()
    gidx = nc.dram_tensor(
        "gidx", [P, TD * NB * T_b * 8], i16, kind="ExternalInput"
    ).ap()
    dstl_bf = nc.dram_tensor(
        "dstl_bf", [P, TD * T_e], bf16, kind="ExternalInput"
    ).ap()
    w1 = nc.dram_tensor("w1", [D_IN, H1], f32, kind="ExternalInput").ap()
    w2 = nc.dram_tensor("w2", [H1, H2], f32, kind="ExternalInput").ap()
    b1c = nc.dram_tensor("b1c", [P, H1 // P], f32, kind="ExternalInput").ap()
    b2c = nc.dram_tensor("b2c", [P, 1], f32, kind="ExternalInput").ap()
    dinv_d = nc.dram_tensor("dinv_d", [P, TD], f32, kind="ExternalInput").ap()
    out = nc.dram_tensor("out", [SHARD, H2], f32, kind="ExternalOutput").ap()

    qn = [0]

    def next_q():
        qn[0] = (qn[0] + 1) % 4
        return qn[0]

    with tile.TileContext(nc) as tc:
        with (
            tc.tile_pool(name="const", bufs=1) as cp,
            tc.tile_pool(name="msg", bufs=16) as mp,
            tc.tile_pool(name="sp", bufs=4) as spp,
            tc.tile_pool(name="work", bufs=3) as wp,
            tc.tile_pool(name="grp", bufs=2) as gp,
            tc.tile_pool(name="psy", bufs=4, space="PSUM") as psy,
            tc.tile_pool(name="pst", bufs=2, space="PSUM") as pst,
            tc.tile_pool(name="psh", bufs=2, space="PSUM") as psh,
            tc.tile_pool(name="dram", bufs=1, space="DRAM") as dp,
        ):
            # ---- constants ----
            iota_i = cp.tile([P, T_e * P], mybir.dt.int32)
            nc.gpsimd.iota(iota_i[:], pattern=[[0, T_e], [1, P]], base=0, channel_multiplier=0)
            iota_bf = cp.tile([P, T_e * P], bf16)
            nc.vector.tensor_copy(iota_bf[:], iota_i[:])
            ident = cp.tile([P, P], f32)
            make_identity(nc, ident[:])

            gidx_t = cp.tile([P, TD * NB * T_b * 8], i16)
            nc.sync.dma_start(gidx_t[:], gidx[:, :])
            dstlb_t = cp.tile([P, TD * T_e], bf16)
            nc.sync.dma_start(dstlb_t[:], dstl_bf[:, :])
            b1_t = cp.tile([P, H1 // P], f32)
            nc.sync.dma_start(b1_t[:], b1c[:, :])
            b2_t = cp.tile([P, 1], f32)
            nc.sync.dma_start(b2_t[:], b2c[:, :])
            dinv_t = cp.tile([P, TD], f32)
            nc.sync.dma_start(dinv_t[:], dinv_d[:, :])
            w1_t = [cp.tile([P, H1], f32r, tag=f"w1_{k}", name=f"w1_{k}") for k in range(2)]
            for k in range(2):
                nc.gpsimd.dma_start(w1_t[k][:], w1[k * P : (k + 1) * P, :])
            w2_t = [cp.tile([P, H2], f32r, tag=f"w2_{k}", name=f"w2_{k}") for k in range(2)]
            for k in range(2):
                nc.gpsimd.dma_start(w2_t[k][:], w2[k * P : (k + 1) * P, :])

            h2_shard = dp.tile([SHARD, H2], bf16)
            h2_full = dp.tile([NPAD, H2], bf16)

            def bcast(ap_tile, d0, n_t):
                a = ap_tile[:, d0 : d0 + n_t]
                return bass.AP(
                    a.tensor, a.offset, [a.ap[0], [a.ap[1][0], n_t], [0, P]]
                )

            # ================= layer 1 + dense =================
            GRP = 4
            ngrp = (TD + GRP - 1) // GRP
            gsizes = [min(GRP, TD - g * GRP) for g in range(ngrp)]
            col_base = [0]
            for g in range(ngrp):
                for b in range(NB):
                    col_base.append(col_base[-1] + gsizes[g] * T_b * 8)
            for g in range(ngrp):
                gs = gsizes[g]
                dlist = list(range(g * GRP, g * GRP + gs))
                yT = [gp.tile([P, GRP * P], f32r, tag=f"yT{h}", name=f"yT{h}") for h in range(2)]
                for di, d in enumerate(dlist):
                    msgs = []
                    for b in range(NB):
                        m = mp.tile([P, T_b, D_IN], bf16, tag="msg1", name="m1")
                        cb = col_base[g * NB + b] + di * T_b * 8
                        nc.gpsimd.dma_gather(
                            out_ap=m[:],
                            in_ap=xs[b * BANK : (b + 1) * BANK, :],
                            idxs_ap=gidx_t[:, cb : cb + T_b * 8],
                            num_idxs=T_b * P,
                            num_idxs_reg=T_b * P,
                            elem_size=D_IN,
                            single_packet=False,
                            queue_num=next_q(),
                        )
                        msgs.append(m)
                    sp = spp.tile([P, T_e * P], bf16, tag="sp1")
                    nc.vector.tensor_tensor(
                        out=sp[:],
                        in0=iota_bf[:].rearrange("p (t q) -> p t q", t=T_e),
                        in1=bcast(dstlb_t, d * T_e, T_e),
                        op=mybir.AluOpType.is_equal,
                    )
                    acc = psy.tile([P, D_IN], f32, tag="acc1")
                    for b in range(NB):
                        for j in range(T_b):
                            t = b * T_b + j
                            nc.tensor.matmul(
                                acc[:],
                                lhsT=sp[:, t * P : (t + 1) * P],
                                rhs=msgs[b][:, j, :],
                                start=(t == 0),
                                stop=(t == T_e - 1),
                            )
                    y_sb = wp.tile([P, D_IN], f32, tag="ysb")
                    nc.vector.tensor_scalar(
                        out=y_sb[:],
                        in0=acc[:],
                        scalar1=dinv_t[:, d : d + 1],
                        scalar2=None,
                        op0=mybir.AluOpType.mult,
                    )
                    for h in range(2):
                        tp = pst.tile([P, P], f32, tag="tp")
                        nc.tensor.transpose(
                            out=tp[:], in_=y_sb[:, h * P : (h + 1) * P], identity=ident[:]
                        )
                        nc.vector.tensor_copy(yT[h][:, di * P : (di + 1) * P], tp[:])
                # dense: h1T = W1^T yT ; rT = relu(h1T + b1); h2T = W2^T rT
                nn = gs * P
                rT = [gp.tile([P, GRP * P], f32r, tag=f"rT{o}", name=f"rT{o}") for o in range(2)]
                for o in range(2):
                    ph1 = psh.tile([P, GRP * P], f32, tag="ph1")
                    for k in range(2):
                        nc.tensor.matmul(
                            ph1[:, :nn],
                            lhsT=w1_t[k][:, o * P : (o + 1) * P],
                            rhs=yT[k][:, :nn],
                            start=(k == 0),
                            stop=(k == 1),
                        )
                    nc.scalar.activation(
                        out=rT[o][:, :nn],
                        in_=ph1[:, :nn],
                        func=mybir.ActivationFunctionType.Relu,
                        bias=b1_t[:, o : o + 1],
                        scale=1.0,
                    )
                ph2 = psh.tile([P, GRP * P], f32, tag="ph1")
                for k in range(2):
                    nc.tensor.matmul(
                        ph2[:, :nn],
                        lhsT=w2_t[k][:, :],
                        rhs=rT[k][:, :nn],
                        start=(k == 0),
                        stop=(k == 1),
                    )
                h2T_sb = wp.tile([P, GRP * P], f32, tag="h2T")
                nc.vector.tensor_copy(h2T_sb[:, :nn], ph2[:, :nn])
                h2_sb = wp.tile([P, GRP, P], bf16, tag="h2sb")
                for qi, d in enumerate(dlist):
                    tp2 = pst.tile([P, P], f32, tag="tp")
                    nc.tensor.transpose(
                        out=tp2[:], in_=h2T_sb[:, qi * P : (qi + 1) * P], identity=ident[:]
                    )
                    nc.vector.tensor_scalar(
                        out=h2_sb[:, qi, :],
                        in0=tp2[:],
                        scalar1=dinv_t[:, d : d + 1],
                        scalar2=None,
                        op0=mybir.AluOpType.mult,
                    )
                dst_rows = h2_shard[
                    dlist[0] * P : (dlist[0] + gs) * P, :
                ].rearrange("(t p) f -> p t f", p=P)
                nc.sync.dma_start(dst_rows, h2_sb[:, :gs, :])

            # ================= exchange =================
            nc.gpsimd.collective_compute(
                "AllGather",
                mybir.AluOpType.bypass,
                ins=[h2_shard.opt()],
                outs=[h2_full.opt()],
                replica_groups=[list(range(NC))],
            )

            # ================= layer 2 =================
            for g in range(ngrp):
                gs = gsizes[g]
                dlist = list(range(g * GRP, g * GRP + gs))
                for di, d in enumerate(dlist):
                    msgs = []
                    for b in range(NB):
                        m = mp.tile([P, T_b, H2], bf16, tag="msg2", name="m2")
                        cb = col_base[g * NB + b] + di * T_b * 8
                        nc.gpsimd.dma_gather(
                            out_ap=m[:],
                            in_ap=h2_full[b * BANK : (b + 1) * BANK, :],
                            idxs_ap=gidx_t[:, cb : cb + T_b * 8],
                            num_idxs=T_b * P,
                            num_idxs_reg=T_b * P,
                            elem_size=H2,
                            single_packet=False,
                            queue_num=next_q(),
                        )
                        msgs.append(m)
                    sp2 = spp.tile([P, T_e * P], bf16, tag="sp2")
                    nc.vector.tensor_tensor(
                        out=sp2[:],
                        in0=iota_bf[:].rearrange("p (t q) -> p t q", t=T_e),
                        in1=bcast(dstlb_t, d * T_e, T_e),
                        op=mybir.AluOpType.is_equal,
                    )
                    acc2 = psy.tile([P, H2], f32, tag="acc1")
                    for b in range(NB):
                        for j in range(T_b):
                            t = b * T_b + j
                            nc.tensor.matmul(
                                acc2[:],
                                lhsT=sp2[:, t * P : (t + 1) * P],
                                rhs=msgs[b][:, j, :],
                                start=(t == 0),
                                stop=(t == T_e - 1),
                            )
                    o_sb = wp.tile([P, H2], f32, tag="osb")
                    nc.vector.tensor_scalar(
                        out=o_sb[:],
                        in0=acc2[:],
                        scalar1=dinv_t[:, d : d + 1],
                        scalar2=b2_t[:, :1],
                        op0=mybir.AluOpType.mult,
                        op1=mybir.AluOpType.add,
                    )
                    nc.sync.dma_start(out[d * P : (d + 1) * P, :], o_sb[:])

    nc.compile()
    return nc


_CACHED = {}


def kernel(x, W1, b1, W2, b2, edge_index):
    global LAST_EXEC_NS, LAST_RESULT, LAST_TB
    x = np.asarray(x, dtype=np.float32)
    W1 = np.asarray(W1, dtype=np.float32)
    b1 = np.asarray(b1, dtype=np.float32)
    W2 = np.asarray(W2, dtype=np.float32)
    b2 = np.asarray(b2, dtype=np.float32)
    ei = np.asarray(edge_index)
    src = ei[0].astype(np.int64)
    dst = ei[1].astype(np.int64)
    n = x.shape[0]
    # self loops
    loop = np.arange(n, dtype=np.int64)
    src_f = np.concatenate([src, loop])
    dst_f = np.concatenate([dst, loop])
    deg = np.bincount(dst_f, minlength=n).astype(np.float32)
    dinv = np.where(deg > 0, 1.0 / np.sqrt(deg), 0.0).astype(np.float32)

    xs = np.zeros((NPAD, D_IN), dtype=ml_dtypes.bfloat16)
    xs[:n] = (x * dinv[:, None]).astype(ml_dtypes.bfloat16)
    dinv_pad = np.zeros(NPAD, dtype=np.float32)
    dinv_pad[:n] = dinv

    T_b, cores = _pack(src_f, dst_f)
    global LAST_TB
    LAST_TB = T_b

    key = T_b
    if key not in _CACHED:
        _CACHED[key] = _build(T_b)
    ncobj = _CACHED[key]

    b1c = b1.reshape(H1 // P, P).T.copy()
    b2c = b2.reshape(1, P).T.copy()
    in_maps = []
    for c in range(NC):
        dinv_d = dinv_pad[c * SHARD : (c + 1) * SHARD].reshape(TD, P).T.copy()
        in_maps.append(
            {
                "xs": xs,
                "gidx": cores[c]["gidx"],
                "dstl_bf": cores[c]["dstl_bf"],
                "w1": W1,
                "w2": W2,
                "b1c": b1c,
                "b2c": b2c,
                "dinv_d": dinv_d,
            }
        )

    trace = os.environ.get("KERNEL_TRACE", "0") == "1"
    if trace:
        try:
            import profhook

            profhook.install()
        except Exception:
            trace = False
    res = bass_utils.run_bass_kernel_spmd(
        ncobj, in_maps, core_ids=list(range(NC)), trace=trace
    )
    LAST_EXEC_NS = res.exec_time_ns
    global LAST_RESULT
    LAST_RESULT = res
    out = np.concatenate([res.results[c]["out"] for c in range(NC)], axis=0)
    return out[:n].astype(np.float32)

